# revision 1
# baseline (speedup 1.0000x reference)
"""Bass/Trainium2 kernel for nn_CHAREncoder: char-level BiLSTM encoder.

Reference computation:
  x = emb[char_ids]                      # [B, W, L, E]
  h_f = LSTM_fwd(x)  final hidden        # [B*W, H]
  h_b = LSTM_bwd(x reversed) final hidden
  out = concat(h_f, h_b)                 # [B, W, 2H]

Sharding: pure data parallel over the B*W = 16384 words -> 2048 words/core
on 8 NeuronCores. Embedding table + LSTM weights replicated.

Device-side design (per core, 2048 words = 16 word-tiles of 128):
 - Embedding gather via indirect DMA from a padded fp16 table
   [512, 64] = [emb(32) | 1.0 | zeros], word-major into SBUF.
 - One multi-block DMA-transpose per 2-step group converts word-major x
   into stationary-ready xbarT blocks [128, 128]: rows 64a..64a+32 hold
   x^T for step 2g+a, row 64a+32 holds ones (bias row).
 - Per step/dir/tile: matmul1 (K=33: x + bias via ones-row against
   [Wih^T; b]) + matmul2 (K=32: h^T against Whh^T replica) accumulate
   gates [128 words, 128 gates] into PSUM.
 - Gate order is permuted to [i, f, o, g] so one sigmoid ACT op covers
   gates 0:96 and one tanh op covers 96:128 (strided over 16 tiles).
 - c kept in fp32, everything else fp16. h is re-transposed each step via
   SBUF->SBUF DMA-transpose to feed the next step's stationary operand.
"""
import numpy as np
import ml_dtypes

import concourse.bass as bass
import concourse.bacc as bacc
import concourse.tile as tile
from concourse import mybir
from concourse._compat import with_exitstack

B, W, L = 64, 256, 25
V, E, H = 399, 32, 32
NCORES = 8
NW = (B * W) // NCORES          # words per core = 2048
NT = NW // 128                  # word tiles per core = 16
NG = (L + 1) // 2               # 2-step transpose groups = 13
VP = 512                        # padded vocab rows
EWID = 64                       # table row width (fp16): emb 32 | 1.0 | zeros

_CACHE = {}


def _build_nc():
    nc = bacc.Bacc("TRN2", target_bir_lowering=False)
    f16 = mybir.dt.float16
    f32 = mybir.dt.float32

    xbar = nc.dram_tensor("xbar", [128, NT, NG, 128], f16, kind="ExternalInput")
    wih2 = {d: nc.dram_tensor(f"wih2{d}", [128, 128], f16, kind="ExternalInput")
            for d in "fb"}
    whh4 = {d: nc.dram_tensor(f"whh4{d}", [128, 128], f16, kind="ExternalInput")
            for d in "fb"}
    out = {d: nc.dram_tensor(f"out{d}", [128, NT, H], f16, kind="ExternalOutput")
           for d in "fb"}

    with tile.TileContext(nc) as tc:
        _emit(tc, nc, xbar, wih2, whh4, out)
    nc.compile()
    return nc


@with_exitstack
def _emit(ctx, tc, nc, xbar, wih2, whh4, out):
    f16 = mybir.dt.float16
    f32 = mybir.dt.float32
    AF = mybir.ActivationFunctionType

    const = ctx.enter_context(tc.tile_pool(name="const", bufs=1))
    xwp = ctx.enter_context(tc.tile_pool(name="xw", bufs=3))
    work = ctx.enter_context(tc.tile_pool(name="work", bufs=2))
    state = ctx.enter_context(tc.tile_pool(name="state", bufs=1))
    psum = ctx.enter_context(tc.tile_pool(name="psum", bufs=1, space="PSUM"))

    wih_sb = {}
    whh_sb = {}
    for d in "fb":
        wih_sb[d] = const.tile([128, 128], f16, tag=f"wih{d}", name=f"wih{d}")
        nc.sync.dma_start(out=wih_sb[d], in_=wih2[d][:, :])
        whh_sb[d] = const.tile([128, 128], f16, tag=f"whh{d}", name=f"whh{d}")
        nc.sync.dma_start(out=whh_sb[d], in_=whh4[d][:, :])

    # xbarT[p, t, g, w]: transposed x blocks; rows 64a..64a+33 of block
    # (t, g) are [x^T; ones] for step s = 2g + a, words t*128..t*128+128.
    # Host performs the embedding gather + transpose; one chunked DMA per
    # group column so the recurrence can start before the full load.
    xbarT = const.tile([128, NT, NG, 128], f16)
    for g in range(NG):
        nc.sync.dma_start(out=xbarT[:, :, g, :], in_=xbar[:, :, g, :])

    c = {d: state.tile([128, NT, H], f32, tag=f"c{d}", name=f"c{d}") for d in "fb"}
    hT_prev = {}

    for k in range(L):
        for d in "fb":
            s = k if d == "f" else (L - 1 - k)
            g, a = divmod(s, 2)
            gates = psum.tile([128, NT, 128], f32, tag=f"gates{d}")
            for t in range(NT):
                nc.tensor.matmul(
                    gates[:, t, :],
                    xbarT[64 * a:64 * a + 33, t, g, :],
                    wih_sb[d][64 * a:64 * a + 33, :],
                    start=True, stop=(k == 0),
                    tile_position=(64 * a, 0),
                )
                if k > 0:
                    tp = 32 * (t % 4)
                    nc.tensor.matmul(
                        gates[:, t, :],
                        hT_prev[d][tp:tp + 32, t // 4, :],
                        whh_sb[d][tp:tp + 32, :],
                        start=False, stop=True,
                        tile_position=(tp, 0),
                    )
            sig = work.tile([128, NT, 128], f16, tag=f"sig{d}")
            nc.scalar.activation(sig[:, :, 0:96], gates[:, :, 0:96], AF.Sigmoid)
            nc.scalar.activation(sig[:, :, 96:128], gates[:, :, 96:128], AF.Tanh)
            if k == 0:
                nc.vector.tensor_mul(c[d], sig[:, :, 0:32], sig[:, :, 96:128])
            else:
                u = work.tile([128, NT, H], f16, tag=f"u{d}")
                nc.vector.tensor_mul(u, sig[:, :, 0:32], sig[:, :, 96:128])
                v = work.tile([128, NT, H], f32, tag=f"v{d}")
                nc.vector.tensor_mul(v, sig[:, :, 32:64], c[d])
                nc.vector.tensor_add(c[d], u, v)
            tc_t = work.tile([128, NT, H], f16, tag=f"tc{d}")
            nc.scalar.activation(tc_t, c[d], AF.Tanh)
            h = work.tile([128, NT, H], f16, tag=f"h{d}")
            nc.vector.tensor_mul(h, sig[:, :, 64:96], tc_t)
            if k < L - 1:
                hT = work.tile([128, 4, 128], f16, tag=f"hT{d}")
                for grp in range(4):
                    nc.sync.dma_start_transpose(
                        out=hT[:, grp, :],
                        in_=h[:, 4 * grp:4 * grp + 4, :].rearrange(
                            "p t j -> p (t j)"),
                    )
                hT_prev[d] = hT
            else:
                nc.sync.dma_start(out=out[d][:, :, :], in_=h)


def _gate_perm():
    # torch gate order i,f,g,o (blocks of H) -> device order i,f,o,g
    p = np.arange(4 * H)
    return np.concatenate([p[0:H], p[H:2 * H], p[3 * H:4 * H], p[2 * H:3 * H]])


def _host_prep(char_ids, emb, w_ih_f, w_hh_f, b_ih_f, b_hh_f,
               w_ih_b, w_hh_b, b_ih_b, b_hh_b):
    f16 = np.float16
    perm = _gate_perm()

    tab = np.zeros((VP, EWID), f16)
    tab[:V, :E] = emb.astype(f16)
    tab[:, E] = 1.0

    def wstack(w_ih, b_ih, b_hh):
        m = np.zeros((128, 128), f16)
        wt = w_ih[perm, :].T.astype(np.float32)          # [32, 128]
        bb = (b_ih + b_hh)[perm].astype(np.float32)      # [128]
        m[0:E, :] = wt.astype(f16)
        m[E, :] = bb.astype(f16)
        m[64:64 + E, :] = wt.astype(f16)
        m[64 + E, :] = bb.astype(f16)
        return m

    def hstack(w_hh):
        m = np.zeros((128, 128), f16)
        wt = w_hh[perm, :].T.astype(f16)                 # [32, 128]
        for kk in range(4):
            m[32 * kk:32 * kk + H, :] = wt
        return m

    ids = np.asarray(char_ids).reshape(B * W, L).astype(np.int32)
    # gather + pad odd step + transpose into stationary blocks, vectorized
    X = tab[ids, :]                                      # [B*W, L, 64]
    Xp = np.zeros((B * W, 2 * NG, EWID), f16)
    Xp[:, :L, :] = X
    in_maps = []
    for core in range(NCORES):
        Xc = Xp[core * NW:(core + 1) * NW]               # [2048, 26, 64]
        # (t, w, g, a, e) -> (a, e, t, g, w)
        xbar = np.ascontiguousarray(
            Xc.reshape(NT, 128, NG, 2, EWID).transpose(3, 4, 0, 2, 1)
        ).reshape(128, NT, NG, 128)
        in_maps.append({
            "xbar": xbar,
            "wih2f": wstack(w_ih_f, b_ih_f, b_hh_f),
            "whh4f": hstack(w_hh_f),
            "wih2b": wstack(w_ih_b, b_ih_b, b_hh_b),
            "whh4b": hstack(w_hh_b),
        })
    return in_maps


def _assemble(results):
    outs = []
    for core in range(NCORES):
        hf = np.asarray(results[core]["outf"], np.float32)   # [128, 16, 32]
        hb = np.asarray(results[core]["outb"], np.float32)
        o = np.concatenate([hf, hb], axis=-1)                # [128, 16, 64]
        outs.append(o.transpose(1, 0, 2).reshape(NW, 2 * H))
    return np.concatenate(outs, axis=0).reshape(B, W, 2 * H)


def _run(inputs, trace=False):
    from concourse.bass_utils import run_bass_kernel_spmd
    if "nc" not in _CACHE:
        _CACHE["nc"] = _build_nc()
    nc = _CACHE["nc"]
    in_maps = _host_prep(**inputs)
    kw = {}
    if trace:
        kw = dict(trace=True)
    res = run_bass_kernel_spmd(nc, in_maps, core_ids=list(range(NCORES)), **kw)
    return _assemble(res.results), res


def kernel(**inputs) -> np.ndarray:
    out, _ = _run(inputs)
    return out



# revision 3
# speedup vs baseline: 2.2546x; 2.2546x over previous
"""Bass/Trainium2 kernel for nn_CHAREncoder: char-level BiLSTM encoder.

Reference computation:
  x = emb[char_ids]                      # [B, W, L, E]
  h_f = LSTM_fwd(x)  final hidden        # [B*W, H]
  h_b = LSTM_bwd(x reversed) final hidden
  out = concat(h_f, h_b)                 # [B, W, 2H]

Sharding: pure data parallel over the B*W = 16384 words -> 2048 words/core
on 8 NeuronCores. Embedding table + LSTM weights replicated.

Key idea: the input projection x_t @ W_ih^T + b only depends on the char id,
so it is precomposed on host into a tiny gates table
  G_d[v, :] = emb[v] @ W_ih_d^T + (b_ih_d + b_hh_d)   # [399, 128] per dir
and the whole input-side work becomes an on-device indirect-DMA row gather
(256 B per word-step). Only char_ids (int32) and the small tables ship to
the device (~0.5 MB/core instead of ~7 MB/core of host-gathered activations),
which is what dominates the end-to-end time through the axon tunnel.

Device-side per step/dir: gather G rows for all 2048 words in one indirect
DMA -> SBUF fp16; pass into PSUM via an identity matmul; accumulate the
recurrent term h_{t-1}^T @ W_hh^T (4x32-row PE quadrants, one per word-tile
group); one sigmoid ACT over gates [i,f,o] and one tanh ACT over [g]
(device gate order i,f,o,g); fp32 cell state; h re-transposed each step via
SBUF->SBUF DMA-transpose to feed the next step's stationary operand.
"""
import numpy as np
import ml_dtypes

import concourse.bass as bass
import concourse.bacc as bacc
import concourse.tile as tile
from concourse import mybir
from concourse.masks import make_identity
from concourse._compat import with_exitstack

B, W, L = 64, 256, 25
V, E, H = 399, 32, 32
NCORES = 8
NW = (B * W) // NCORES          # words per core = 2048
NT = NW // 128                  # word tiles per core = 16
VP = 512                        # padded vocab rows in the gates tables

_CACHE = {}


def _build_nc():
    nc = bacc.Bacc("TRN2", target_bir_lowering=False)
    f16 = mybir.dt.float16
    i32 = mybir.dt.int32

    ids = nc.dram_tensor("ids", [128, L, NT], i32, kind="ExternalInput")
    gtab = {d: nc.dram_tensor(f"gtab{d}", [VP, 128], f16, kind="ExternalInput")
            for d in "fb"}
    whh4 = {d: nc.dram_tensor(f"whh4{d}", [128, 128], f16, kind="ExternalInput")
            for d in "fb"}
    out = {d: nc.dram_tensor(f"out{d}", [128, NT, H], f16, kind="ExternalOutput")
           for d in "fb"}

    with tile.TileContext(nc) as tc:
        _emit(tc, nc, ids, gtab, whh4, out)
    nc.compile()
    return nc


@with_exitstack
def _emit(ctx, tc, nc, ids, gtab, whh4, out):
    f16 = mybir.dt.float16
    f32 = mybir.dt.float32
    i32 = mybir.dt.int32
    AF = mybir.ActivationFunctionType

    const = ctx.enter_context(tc.tile_pool(name="const", bufs=1))
    gbp = ctx.enter_context(tc.tile_pool(name="gb", bufs=3))
    work = ctx.enter_context(tc.tile_pool(name="work", bufs=2))
    state = ctx.enter_context(tc.tile_pool(name="state", bufs=1))
    psum = ctx.enter_context(tc.tile_pool(name="psum", bufs=1, space="PSUM"))

    ids_sb = const.tile([128, L, NT], i32, tag="ids", name="ids_sb")
    nc.sync.dma_start(out=ids_sb, in_=ids[:, :, :])

    whh_sb = {}
    for d in "fb":
        whh_sb[d] = const.tile([128, 128], f16, tag=f"whh{d}", name=f"whh{d}")
        nc.sync.dma_start(out=whh_sb[d], in_=whh4[d][:, :])

    ident = const.tile([128, 128], f16, tag="ident", name="ident")
    make_identity(nc, ident)

    c = {d: state.tile([128, NT, H], f32, tag=f"c{d}", name=f"c{d}") for d in "fb"}
    hT_prev = {}

    for k in range(L):
        for d in "fb":
            s = k if d == "f" else (L - 1 - k)
            # gather gates-table rows for step s, one [128,1]-indexed
            # gather per word tile (multi-column index APs misbehave on HW)
            gb = gbp.tile([128, NT, 128], f16, tag=f"gb{d}")
            for t in range(NT):
                nc.gpsimd.indirect_dma_start(
                    out=gb[:, t, :],
                    out_offset=None,
                    in_=gtab[d][:, :],
                    in_offset=bass.IndirectOffsetOnAxis(
                        ap=ids_sb[:, s, t:t + 1], axis=0),
                )
            gates = psum.tile([128, NT, 128], f32, tag=f"gates{d}")
            for t in range(NT):
                # identity matmul passes the gathered input projection into
                # the PSUM accumulation group
                nc.tensor.matmul(
                    gates[:, t, :],
                    ident[:, :],
                    gb[:, t, :],
                    start=True, stop=(k == 0),
                )
                if k > 0:
                    tp = 32 * (t % 4)
                    nc.tensor.matmul(
                        gates[:, t, :],
                        hT_prev[d][tp:tp + 32, t // 4, :],
                        whh_sb[d][tp:tp + 32, :],
                        start=False, stop=True,
                        tile_position=(tp, 0),
                    )
            sig = work.tile([128, NT, 128], f16, tag=f"sig{d}")
            nc.scalar.activation(sig[:, :, 0:96], gates[:, :, 0:96], AF.Sigmoid)
            nc.scalar.activation(sig[:, :, 96:128], gates[:, :, 96:128], AF.Tanh)
            if k == 0:
                nc.vector.tensor_mul(c[d], sig[:, :, 0:32], sig[:, :, 96:128])
            else:
                u = work.tile([128, NT, H], f16, tag=f"u{d}")
                nc.vector.tensor_mul(u, sig[:, :, 0:32], sig[:, :, 96:128])
                v = work.tile([128, NT, H], f32, tag=f"v{d}")
                nc.vector.tensor_mul(v, sig[:, :, 32:64], c[d])
                nc.vector.tensor_add(c[d], u, v)
            tc_t = work.tile([128, NT, H], f16, tag=f"tc{d}")
            nc.scalar.activation(tc_t, c[d], AF.Tanh)
            h = work.tile([128, NT, H], f16, tag=f"h{d}")
            nc.vector.tensor_mul(h, sig[:, :, 64:96], tc_t)
            if k < L - 1:
                hT = work.tile([128, 4, 128], f16, tag=f"hT{d}")
                for grp in range(4):
                    nc.sync.dma_start_transpose(
                        out=hT[:, grp, :],
                        in_=h[:, 4 * grp:4 * grp + 4, :].rearrange(
                            "p t j -> p (t j)"),
                    )
                hT_prev[d] = hT
            else:
                nc.sync.dma_start(out=out[d][:, :, :], in_=h)


def _gate_perm():
    # torch gate order i,f,g,o (blocks of H) -> device order i,f,o,g
    p = np.arange(4 * H)
    return np.concatenate([p[0:H], p[H:2 * H], p[3 * H:4 * H], p[2 * H:3 * H]])


def _host_prep(char_ids, emb, w_ih_f, w_hh_f, b_ih_f, b_hh_f,
               w_ih_b, w_hh_b, b_ih_b, b_hh_b):
    f16 = np.float16
    perm = _gate_perm()

    def gtab(w_ih, b_ih, b_hh):
        # precomposed input projection: G[v, :] = emb[v] @ W_ih^T + b
        g = emb.astype(np.float32) @ w_ih[perm, :].T.astype(np.float32)
        g += (b_ih + b_hh)[perm].astype(np.float32)
        m = np.zeros((VP, 128), f16)
        m[:V, :] = g.astype(f16)
        return m

    def hstack(w_hh):
        m = np.zeros((128, 128), f16)
        wt = w_hh[perm, :].T.astype(f16)                 # [32, 128]
        for kk in range(4):
            m[32 * kk:32 * kk + H, :] = wt
        return m

    gf = gtab(w_ih_f, b_ih_f, b_hh_f)
    gbt = gtab(w_ih_b, b_ih_b, b_hh_b)
    whhf = hstack(w_hh_f)
    whhb = hstack(w_hh_b)

    # ids[c][p, s, t] = char id of word (c*NW + t*128 + p) at position s
    ids = np.ascontiguousarray(
        np.asarray(char_ids).reshape(NCORES, NT, 128, L)
        .astype(np.int32).transpose(0, 2, 3, 1)
    )
    return [{
        "ids": ids[core],
        "gtabf": gf,
        "gtabb": gbt,
        "whh4f": whhf,
        "whh4b": whhb,
    } for core in range(NCORES)]


def _assemble(results):
    outs = []
    for core in range(NCORES):
        hf = np.asarray(results[core]["outf"], np.float32)   # [128, 16, 32]
        hb = np.asarray(results[core]["outb"], np.float32)
        o = np.concatenate([hf, hb], axis=-1)                # [128, 16, 64]
        outs.append(o.transpose(1, 0, 2).reshape(NW, 2 * H))
    return np.concatenate(outs, axis=0).reshape(B, W, 2 * H)


def _run(inputs, trace=False):
    from concourse.bass_utils import run_bass_kernel_spmd
    if "nc" not in _CACHE:
        _CACHE["nc"] = _build_nc()
    nc = _CACHE["nc"]
    in_maps = _host_prep(**inputs)
    kw = {}
    if trace:
        kw = dict(trace=True)
    res = run_bass_kernel_spmd(nc, in_maps, core_ids=list(range(NCORES)), **kw)
    return _assemble(res.results), res


def kernel(**inputs) -> np.ndarray:
    out, _ = _run(inputs)
    return out


# revision 8
# speedup vs baseline: 6.6103x; 2.9319x over previous
"""Bass/Trainium2 kernel for nn_CHAREncoder: char-level BiLSTM encoder.

Reference computation:
  x = emb[char_ids]                      # [B, W, L, E]
  h_f = LSTM_fwd(x)  final hidden        # [B*W, H]
  h_b = LSTM_bwd(x reversed) final hidden
  out = concat(h_f, h_b)                 # [B, W, 2H]

Sharding: pure data parallel over the B*W = 16384 words -> 2048 words/core
on 8 NeuronCores. Embedding table + LSTM weights replicated.

Key idea: the input projection x_t @ W_ih^T + b only depends on the char id,
so it is precomposed on host into a tiny gates table
  G_d[v, :] = emb[v] @ W_ih_d^T + (b_ih_d + b_hh_d)   # [399, 128] per dir
and the whole input-side work becomes an on-device indirect-DMA row gather
(256 B per word-step). Only char_ids (int32) and the small tables ship to
the device (~0.5 MB/core instead of ~7 MB/core of host-gathered activations),
which is what dominates the end-to-end time through the axon tunnel.

Device-side per step/dir: gather G rows for all 2048 words in one indirect
DMA -> SBUF fp16; pass into PSUM via an identity matmul; accumulate the
recurrent term h_{t-1}^T @ W_hh^T (4x32-row PE quadrants, one per word-tile
group); one sigmoid ACT over gates [i,f,o] and one tanh ACT over [g]
(device gate order i,f,o,g); fp32 cell state; h re-transposed each step via
SBUF->SBUF DMA-transpose to feed the next step's stationary operand.
"""
import numpy as np
import ml_dtypes

import jax

# Persistent XLA compilation cache: the per-call jax.jit inside
# run_bass_kernel_spmd re-lowers the identical HLO every call; with the
# cache enabled the BIR->NEFF backend compile (~0.5 s) is skipped on all
# but the very first call on this machine.
try:
    jax.config.update("jax_compilation_cache_dir", "/tmp/jax_comp_cache")
    jax.config.update("jax_persistent_cache_min_compile_time_secs", 0.0)
    jax.config.update("jax_persistent_cache_min_entry_size_bytes", -1)
except Exception:
    pass

import concourse.bass as bass
import concourse.bacc as bacc
import concourse.tile as tile
from concourse import mybir
from concourse.masks import make_identity
from concourse._compat import with_exitstack

B, W, L = 64, 256, 25
V, E, H = 399, 32, 32
NCORES = 8
NW = (B * W) // NCORES          # words per core = 2048
NT = NW // 128                  # word tiles per core = 16
VP = 512                        # padded vocab rows in the gates tables

_CACHE = {}


def _build_nc():
    nc = bacc.Bacc("TRN2", target_bir_lowering=False)
    f16 = mybir.dt.float16
    i32 = mybir.dt.int32

    ids = nc.dram_tensor("ids", [128, L, NT], i32, kind="ExternalInput")
    gtab = {d: nc.dram_tensor(f"gtab{d}", [VP, 128], f16, kind="ExternalInput")
            for d in "fb"}
    whh4 = {d: nc.dram_tensor(f"whh4{d}", [128, 128], f16, kind="ExternalInput")
            for d in "fb"}
    # single output tensor (fwd hidden in [:, :, 0:H], bwd in [:, :, H:2H]):
    # one sharded D2H fetch instead of two (~70 ms fixed cost per array)
    out = nc.dram_tensor("out", [128, NT, 2 * H], f16, kind="ExternalOutput")

    with tile.TileContext(nc) as tc:
        _emit(tc, nc, ids, gtab, whh4, out)
    nc.compile()
    return nc


@with_exitstack
def _emit(ctx, tc, nc, ids, gtab, whh4, out):
    # out: [128, NT, 2H] fp16, fwd hidden in columns 0:H, bwd in H:2H
    f16 = mybir.dt.float16
    f32 = mybir.dt.float32
    i32 = mybir.dt.int32
    AF = mybir.ActivationFunctionType

    const = ctx.enter_context(tc.tile_pool(name="const", bufs=1))
    gbp = ctx.enter_context(tc.tile_pool(name="gb", bufs=3))
    work = ctx.enter_context(tc.tile_pool(name="work", bufs=2))
    state = ctx.enter_context(tc.tile_pool(name="state", bufs=1))
    psum = ctx.enter_context(tc.tile_pool(name="psum", bufs=1, space="PSUM"))

    ids_sb = const.tile([128, L, NT], i32, tag="ids", name="ids_sb")
    nc.sync.dma_start(out=ids_sb, in_=ids[:, :, :])

    whh_sb = {}
    for d in "fb":
        whh_sb[d] = const.tile([128, 128], f16, tag=f"whh{d}", name=f"whh{d}")
        nc.sync.dma_start(out=whh_sb[d], in_=whh4[d][:, :])

    ident = const.tile([128, 128], f16, tag="ident", name="ident")
    make_identity(nc, ident)

    c = {d: state.tile([128, NT, H], f32, tag=f"c{d}", name=f"c{d}") for d in "fb"}
    hT_prev = {}

    for k in range(L):
        for d in "fb":
            s = k if d == "f" else (L - 1 - k)
            # gather gates-table rows for step s, one [128,1]-indexed
            # gather per word tile (multi-column index APs misbehave on HW)
            gb = gbp.tile([128, NT, 128], f16, tag=f"gb{d}")
            for t in range(NT):
                nc.gpsimd.indirect_dma_start(
                    out=gb[:, t, :],
                    out_offset=None,
                    in_=gtab[d][:, :],
                    in_offset=bass.IndirectOffsetOnAxis(
                        ap=ids_sb[:, s, t:t + 1], axis=0),
                )
            gates = psum.tile([128, NT, 128], f32, tag=f"gates{d}")
            for t in range(NT):
                # identity matmul passes the gathered input projection into
                # the PSUM accumulation group
                nc.tensor.matmul(
                    gates[:, t, :],
                    ident[:, :],
                    gb[:, t, :],
                    start=True, stop=(k == 0),
                )
                if k > 0:
                    tp = 32 * (t % 4)
                    nc.tensor.matmul(
                        gates[:, t, :],
                        hT_prev[d][tp:tp + 32, t // 4, :],
                        whh_sb[d][tp:tp + 32, :],
                        start=False, stop=True,
                        tile_position=(tp, 0),
                    )
            sig = work.tile([128, NT, 128], f16, tag=f"sig{d}")
            nc.scalar.activation(sig[:, :, 0:96], gates[:, :, 0:96], AF.Sigmoid)
            nc.scalar.activation(sig[:, :, 96:128], gates[:, :, 96:128], AF.Tanh)
            if k == 0:
                nc.vector.tensor_mul(c[d], sig[:, :, 0:32], sig[:, :, 96:128])
            else:
                u = work.tile([128, NT, H], f16, tag=f"u{d}")
                nc.vector.tensor_mul(u, sig[:, :, 0:32], sig[:, :, 96:128])
                v = work.tile([128, NT, H], f32, tag=f"v{d}")
                nc.vector.tensor_mul(v, sig[:, :, 32:64], c[d])
                nc.vector.tensor_add(c[d], u, v)
            tc_t = work.tile([128, NT, H], f16, tag=f"tc{d}")
            nc.scalar.activation(tc_t, c[d], AF.Tanh)
            h = work.tile([128, NT, H], f16, tag=f"h{d}")
            nc.vector.tensor_mul(h, sig[:, :, 64:96], tc_t)
            if k < L - 1:
                hT = work.tile([128, 4, 128], f16, tag=f"hT{d}")
                for grp in range(4):
                    nc.sync.dma_start_transpose(
                        out=hT[:, grp, :],
                        in_=h[:, 4 * grp:4 * grp + 4, :].rearrange(
                            "p t j -> p (t j)"),
                    )
                hT_prev[d] = hT
            else:
                off = 0 if d == "f" else H
                nc.sync.dma_start(out=out[:, :, off:off + H], in_=h)


def _gate_perm():
    # torch gate order i,f,g,o (blocks of H) -> device order i,f,o,g
    p = np.arange(4 * H)
    return np.concatenate([p[0:H], p[H:2 * H], p[3 * H:4 * H], p[2 * H:3 * H]])


def _host_prep(char_ids, emb, w_ih_f, w_hh_f, b_ih_f, b_hh_f,
               w_ih_b, w_hh_b, b_ih_b, b_hh_b):
    f16 = np.float16
    perm = _gate_perm()

    def gtab(w_ih, b_ih, b_hh):
        # precomposed input projection: G[v, :] = emb[v] @ W_ih^T + b
        g = emb.astype(np.float32) @ w_ih[perm, :].T.astype(np.float32)
        g += (b_ih + b_hh)[perm].astype(np.float32)
        m = np.zeros((VP, 128), f16)
        m[:V, :] = g.astype(f16)
        return m

    def hstack(w_hh):
        m = np.zeros((128, 128), f16)
        wt = w_hh[perm, :].T.astype(f16)                 # [32, 128]
        for kk in range(4):
            m[32 * kk:32 * kk + H, :] = wt
        return m

    gf = gtab(w_ih_f, b_ih_f, b_hh_f)
    gbt = gtab(w_ih_b, b_ih_b, b_hh_b)
    whhf = hstack(w_hh_f)
    whhb = hstack(w_hh_b)

    # ids[c][p, s, t] = char id of word (c*NW + t*128 + p) at position s
    ids = np.ascontiguousarray(
        np.asarray(char_ids).reshape(NCORES, NT, 128, L)
        .astype(np.int32).transpose(0, 2, 3, 1)
    )
    return [{
        "ids": ids[core],
        "gtabf": gf,
        "gtabb": gbt,
        "whh4f": whhf,
        "whh4b": whhb,
    } for core in range(NCORES)]


def _assemble(results):
    outs = []
    for core in range(NCORES):
        o = np.asarray(results[core]["out"], np.float32)     # [128, 16, 64]
        outs.append(o.transpose(1, 0, 2).reshape(NW, 2 * H))
    return np.concatenate(outs, axis=0).reshape(B, W, 2 * H)


def _run(inputs, trace=False):
    from concourse.bass_utils import run_bass_kernel_spmd
    if "nc" not in _CACHE:
        _CACHE["nc"] = _build_nc()
    nc = _CACHE["nc"]
    in_maps = _host_prep(**inputs)
    kw = {}
    if trace:
        kw = dict(trace=True)
    res = run_bass_kernel_spmd(nc, in_maps, core_ids=list(range(NCORES)), **kw)
    return _assemble(res.results), res


def kernel(**inputs) -> np.ndarray:
    out, _ = _run(inputs)
    return out
